# revision 27
# baseline (speedup 1.0000x reference)
"""Trainium2 Bass kernel for nn_DCGRU (EEG DCGRU: ChebConv+GCN -> biGRU ->
attention -> classifier).

Strategy:
  * Host-side algebraic fold: because F_IN=1, the entire front end
    (ChebConv + GCNConv + node-flatten + GRU input projection) collapses to
    one [192, 64] matrix per direction applied to x[b, :, t], plus a
    constant. This removes the 420MB gru_in intermediate exactly.
  * Data-parallel over batch: 8 cores x 8 batches.
  * Device: stage-1 matmuls (fp32) produce per-step gate preactivations,
    cast to bf16; a fused 200-step bidirectional GRU scan where all scan
    matmuls are bf16 single-pass (fp32 would run 2 LOW/HIGH passes on PE);
    one fused PE inject per step preloads the whole PSUM gate tile.
  * The scan's serial chain per step is mm -> sigmoid -> scan -> tanh ->
    scan; DVE instructions are emitted chain-first so in-order engines
    don't delay the critical path.
  * Stage-1 chunks are emitted interleaved with scan step groups so their
    PE work hides in the scan's idle PE slots.
"""

import numpy as np
from ml_dtypes import bfloat16

N = 64
T = 200
B = 64
H = 64
NC = 8
NB = B // NC          # batches per core
NP = 2 * NB           # scan pair-columns per step (fwd 0:8, bwd 8:16)
TB = T * NB           # stage-1 columns (t, b)
XW = 2 * NP           # 32: per-step Xall columns (16 rz + 16 xn)

_CACHE = {}


def _layout():
    """fp32 blob column layout."""
    off = 0
    bo = {}
    for name, w in (("ab", 1), ("ncb", 1)):
        bo[name] = off
        off += w
    return bo, off


def _layoutx():
    """bf16 x blob column layout."""
    off = 0
    bo = {}
    for name, w in (("xf", TB), ("xb", TB)):
        bo[name] = off
        off += w
    return bo, off


def _layout16():
    """bf16 blob column layout."""
    off = 0
    bo = {}
    for name, w in (
        ("ident", 128), ("Wrzf", 2 * H), ("Wrzb", 2 * H),
        ("Wnf", H), ("Wnb", H), ("attn", 2), ("clf", 2), ("ones", 128),
        ("MfT", 3 * H), ("MbT", 3 * H),
    ):
        bo[name] = off
        off += w
    return bo, off


# --------------------------------------------------------------------------
# host-side fold
# --------------------------------------------------------------------------

def _fold_direction(Wih, bih, Whh, bhh, L, Sg, Wcheb, bcheb, Wgcn, bgcn):
    f64 = np.float64
    Wr = Wih.astype(f64).reshape(3 * H, N, 2 * H)
    Wc = Wr[:, :, 0:H]          # cheb half
    Wg_ = Wr[:, :, H : 2 * H]   # gcn half
    A0 = np.einsum("gnc,c->gn", Wc, Wcheb[0, 0].astype(f64))
    A1 = np.einsum("gnc,c->gn", Wc, Wcheb[1, 0].astype(f64))
    A2 = np.einsum("gnc,c->gn", Wc, Wcheb[2, 0].astype(f64))
    Ag = np.einsum("gnc,c->gn", Wg_, Wgcn[:, :].astype(f64)[0])
    M = A0 + A1 @ L + A2 @ (2.0 * (L @ L) - np.eye(N)) + Ag @ Sg
    cst = (
        np.einsum("gnc,c->g", Wc, bcheb.astype(f64))
        + np.einsum("gnc,c->g", Wg_, bgcn.astype(f64))
        + bih.astype(f64)
    )
    cfull = cst.copy()
    cfull[0 : 2 * H] += bhh.astype(f64)[0 : 2 * H]   # r,z recurrent biases
    MT_aug = np.vstack([M.T, cfull[None, :]]).astype(np.float32)       # [65,192]
    WhT_rz = np.ascontiguousarray(Whh[0 : 2 * H, :].T).astype(np.float32)  # [64,128]
    # negate the z-gate so sigmoid((-u_z)) = 1-z comes out of the one
    # sigmoid directly (removes a per-step cross-partition ACT)
    MT_aug[:, H : 2 * H] *= -1.0
    WhT_rz[:, H : 2 * H] *= -1.0
    WhT_n = np.vstack(
        [Whh[2 * H : 3 * H, :].T, bhh[2 * H : 3 * H][None, :]]
    ).astype(np.float32)                                                # [65,64]
    return MT_aug, WhT_rz, WhT_n


def _fold(inputs):
    f64 = np.float64
    # ChebConv normalized operator (PyG sym norm, lambda_max=2)
    row, col = np.asarray(inputs["spatial_ei"][0]), np.asarray(inputs["spatial_ei"][1])
    ew = np.asarray(inputs["spatial_ew"]).astype(f64)
    deg = np.zeros(N, f64)
    np.add.at(deg, row, ew)
    dinv = np.where(deg > 0, 1.0 / np.sqrt(np.where(deg > 0, deg, 1.0)), 0.0)
    wn = dinv[row] * ew * dinv[col]
    S = np.zeros((N, N), f64)
    np.add.at(S, (col, row), wn)
    L = -S

    # GCNConv operator (gcn_norm with self loops, weight 1)
    row, col = (
        np.asarray(inputs["functional_ei"][0]),
        np.asarray(inputs["functional_ei"][1]),
    )
    ew = np.asarray(inputs["functional_ew"]).astype(f64)
    deg = np.zeros(N, f64)
    np.add.at(deg, col, ew)
    deg += 1.0
    dinv = 1.0 / np.sqrt(deg)
    wn = dinv[row] * ew * dinv[col]
    Sg = np.zeros((N, N), f64)
    np.add.at(Sg, (col, row), wn)
    Sg[np.arange(N), np.arange(N)] += dinv * dinv

    Wcheb = np.asarray(inputs["Wcheb"])
    bcheb = np.asarray(inputs["bcheb"])
    Wgcn = np.asarray(inputs["Wgcn"])
    bgcn = np.asarray(inputs["bgcn"])

    MfT, WhT_rz_f, WhT_n_f = _fold_direction(
        np.asarray(inputs["Wih_f"]), np.asarray(inputs["bih_f"]),
        np.asarray(inputs["Whh_f"]), np.asarray(inputs["bhh_f"]),
        L, Sg, Wcheb, bcheb, Wgcn, bgcn,
    )
    MbT, WhT_rz_b, WhT_n_b = _fold_direction(
        np.asarray(inputs["Wih_b"]), np.asarray(inputs["bih_b"]),
        np.asarray(inputs["Whh_b"]), np.asarray(inputs["bhh_b"]),
        L, Sg, Wcheb, bcheb, Wgcn, bgcn,
    )

    attn_W = np.asarray(inputs["attn_W"]).astype(np.float32)
    clf_W = np.asarray(inputs["clf_W"]).astype(np.float32)
    attn_w2 = np.ascontiguousarray(np.stack([attn_W[0:H, 0], attn_W[H : 2 * H, 0]], 1))
    clf_w2 = np.ascontiguousarray(np.stack([clf_W[0:H, 0], clf_W[H : 2 * H, 0]], 1))
    attn_b = float(np.asarray(inputs["attn_b"]).reshape(-1)[0])
    clf_b = float(np.asarray(inputs["clf_b"]).reshape(-1)[0])

    BO, CB = _layout()
    base = np.zeros((128, CB), np.float32)
    base[0, BO["ab"]] = attn_b
    base[0, BO["ncb"]] = -clf_b

    B16, CB16 = _layout16()
    b16 = np.zeros((128, CB16), np.float32)
    b16[0:128, B16["ident"] : B16["ident"] + 128] = np.eye(128)
    b16[0:H, B16["Wrzf"] : B16["Wrzf"] + 2 * H] = WhT_rz_f
    b16[0:H, B16["Wrzb"] : B16["Wrzb"] + 2 * H] = WhT_rz_b
    b16[0 : H + 1, B16["Wnf"] : B16["Wnf"] + H] = WhT_n_f
    b16[0 : H + 1, B16["Wnb"] : B16["Wnb"] + H] = WhT_n_b
    b16[0:H, B16["attn"] : B16["attn"] + 2] = attn_w2
    b16[0:H, B16["clf"] : B16["clf"] + 2] = clf_w2
    b16[0:1, B16["ones"] : B16["ones"] + 128] = 1.0
    b16[0 : N + 1, B16["MfT"] : B16["MfT"] + 3 * H] = MfT
    b16[0 : N + 1, B16["MbT"] : B16["MbT"] + 3 * H] = MbT
    b16 = b16.astype(bfloat16)
    ones_row = np.ones((1, 32 * (T + 1)), dtype=bfloat16)

    x = np.asarray(inputs["x"]).astype(np.float32)
    XO, CBX = _layoutx()
    in_maps = []
    for c in range(NC):
        xc = x[c * NB : (c + 1) * NB]                       # [NB, N, T]
        xblob = np.zeros((128, CBX), np.float32)
        xblob[0:N, XO["xf"] : XO["xf"] + TB] = xc.transpose(1, 2, 0).reshape(N, TB)
        xblob[N, XO["xf"] : XO["xf"] + TB] = 1.0
        xblob[0:N, XO["xb"] : XO["xb"] + TB] = (
            xc[:, :, ::-1].transpose(1, 2, 0).reshape(N, TB)
        )
        xblob[N, XO["xb"] : XO["xb"] + TB] = 1.0
        xb16 = xblob.astype(bfloat16)
        in_maps.append({
            "blob": base, "blob16": b16,
            "xblobf": np.ascontiguousarray(xb16[:, XO["xf"] : XO["xf"] + TB]),
            "xblobb": np.ascontiguousarray(xb16[:, XO["xb"] : XO["xb"] + TB]),
            "ones": ones_row,
        })
    return in_maps, attn_b, clf_b


# --------------------------------------------------------------------------
# device program
# --------------------------------------------------------------------------

def _build(attn_b: float, clf_b: float):
    import concourse.bass as bass
    import concourse.tile as tile
    from concourse import mybir

    F32 = mybir.dt.float32
    BF16 = mybir.dt.bfloat16
    AF = mybir.ActivationFunctionType
    OP = mybir.AluOpType

    nc = bass.Bass()

    BO, CB = _layout()
    B16, CB16 = _layout16()
    XO, CBX = _layoutx()
    d_blob = nc.declare_dram_parameter("blob", [128, CB], F32, isOutput=False)
    d_b16 = nc.declare_dram_parameter("blob16", [128, CB16], BF16, isOutput=False)
    d_xf = nc.declare_dram_parameter("xblobf", [128, TB], BF16, isOutput=False)
    d_xb = nc.declare_dram_parameter("xblobb", [128, TB], BF16, isOutput=False)
    d_ones = nc.declare_dram_parameter("ones", [1, 32 * (T + 1)], BF16,
                                       isOutput=False)
    d_out = nc.declare_dram_parameter("out", [1, NB], F32, isOutput=True)

    CH = 4                 # stage-1 / attention chunks
    CW = TB // CH          # 400 columns per chunk
    CS = T // CH           # 50 steps per chunk

    with tile.TileContext(nc) as tc:
        with (
            tc.tile_pool(name="const", bufs=1) as cp,
            tc.tile_pool(name="work", bufs=1) as wp,
        ):
            # ---- persistent SBUF tiles
            blob = cp.tile([128, CB], F32)
            b16 = cp.tile([128, CB16], BF16)
            xtf = cp.tile([128, TB], BF16)
            xtb = cp.tile([128, TB], BF16)
            xf = xtf[0 : N + 1, 0:TB]
            xb = xtb[0 : N + 1, 0:TB]
            MfT = b16[0 : N + 1, B16["MfT"] : B16["MfT"] + 3 * H]
            MbT = b16[0 : N + 1, B16["MbT"] : B16["MbT"] + 3 * H]
            ab_t = blob[0:1, BO["ab"] : BO["ab"] + 1]
            ncb_t = blob[0:1, BO["ncb"] : BO["ncb"] + 1]
            Xall = cp.tile([128, XW * T], BF16)
            Hist = cp.tile([H + 1, 32 * (T + 1)], BF16)
            HistB = cp.tile([H, NB * T], BF16)
            ident = b16[0:128, B16["ident"] : B16["ident"] + 128]
            Wrzf = b16[0:H, B16["Wrzf"] : B16["Wrzf"] + 2 * H]
            Wrzb = b16[0:H, B16["Wrzb"] : B16["Wrzb"] + 2 * H]
            Wnf = b16[0 : H + 1, B16["Wnf"] : B16["Wnf"] + H]
            Wnb = b16[0 : H + 1, B16["Wnb"] : B16["Wnb"] + H]
            attn_w = b16[0:H, B16["attn"] : B16["attn"] + 2]
            clf_w = b16[0:H, B16["clf"] : B16["clf"] + 2]
            ones1 = b16[0:1, B16["ones"] : B16["ones"] + 128]

            d0n = cp.tile([128, 2 * NP], F32)      # (0 | r) rows 0:64; (. | z) 64:128
            d0t = cp.tile([H, 2 * NP], F32)        # (0 | 1-z)
            d1t = cp.tile([H, 2 * NP], F32)        # (n | z*h)

            ones_n = wp.tile([1, 128], F32)
            warm16 = wp.tile([H, 128], BF16)

            nc.sync.dma_start(b16[:], d_b16[:])
            nc.sync.dma_start(blob[:], d_blob[:])
            nc.sync.dma_start(xtf[:], d_xf[:])
            nc.gpsimd.dma_start(xtb[:], d_xb[:])
            nc.scalar.dma_start(Hist[H : H + 1, :], d_ones[:])

            # warm the PE HAM clock gate during the DMA: ~5us of matmuls
            # on a scratch psum (never read)
            nc.vector.memset(warm16[:], 1.0)
            tblw = wp.tile([1, 16], F32)
            nc.scalar.activation(tblw[:], warm16[0:1, 0:16], AF.Sigmoid)
            with tc.tile_pool(name="warm", bufs=1, space="PSUM") as pw:
                wps = pw.tile([128, 128], F32)
                for _ in range(14):
                    nc.tensor.matmul(
                        wps[:], warm16[:], warm16[:],
                        start=True, stop=True, skip_group_check=True,
                    )

            nc.vector.memset(Hist[0:H, 0:32], 0.0)
            nc.vector.memset(d0n[:], 0.0)
            nc.vector.memset(d0t[:], 0.0)
            nc.vector.memset(d1t[:], 0.0)
            nc.vector.memset(ones_n[:], 1.0)

            X3 = Xall[:].rearrange("p (i c) -> p i c", c=XW)

            # ---- stage 1 (fp32 math, bf16 out), emitted per-chunk so it can
            # interleave with the scan
            def stage1_chunk(ps1, ch):
                for xa, MT, off in ((xf, MfT, 0), (xb, MbT, NB)):
                    prz = ps1.tile([128, CW], F32)
                    pn = ps1.tile([H, CW], F32)
                    nc.tensor.matmul(
                        prz[0:H, :], MT[:, 0:H],
                        xa[:, ch * CW : (ch + 1) * CW],
                        start=True, stop=True,
                    )
                    nc.tensor.matmul(
                        prz[H:128, :], MT[:, H : 2 * H],
                        xa[:, ch * CW : (ch + 1) * CW],
                        start=True, stop=True, tile_position=(0, 64),
                    )
                    nc.tensor.matmul(
                        pn[:], MT[:, 2 * H : 3 * H],
                        xa[:, ch * CW : (ch + 1) * CW],
                        start=True, stop=True,
                    )
                    src_rz = prz[:].rearrange("p (i c) -> p i c", c=NB)
                    dst_rz = X3[:, ch * CS : (ch + 1) * CS, off : off + NB]
                    nc.vector.tensor_copy(dst_rz, src_rz)
                    src_n = pn[:].rearrange("p (i c) -> p i c", c=NB)
                    dst_n = X3[
                        0:H, ch * CS : (ch + 1) * CS,
                        NP + 2 * off + 1 : NP + 2 * off + 2 * NB : 2,
                    ]
                    nc.vector.tensor_copy(dst_n, src_n)

            # ---- one GRU scan step
            def scan_step(ps2, i):
                hf = Hist[0:H, 32 * i + 1 : 32 * i + 16 : 2]
                hb = Hist[0:H, 32 * i + 17 : 32 * i + 32 : 2]
                hnf = Hist[0 : H + 1, 32 * i + 1 : 32 * i + 16 : 2]
                hnb = Hist[0 : H + 1, 32 * i + 17 : 32 * i + 32 : 2]

                p_rz = ps2.tile([128, NP], F32)
                pnsc = ps2.tile([H, 4 * NP], F32)
                p_n = pnsc[:, 0 : 2 * NP]
                sc = pnsc[:, 2 * NP : 4 * NP]

                nc.tensor.matmul(
                    p_rz[:], ident[:], Xall[:, XW * i : XW * i + NP],
                    start=True, stop=False, skip_group_check=True,
                )
                nc.tensor.matmul(
                    p_n[:, 1 : 2 * NP : 2], ident[0:H, 0:H],
                    Xall[0:H, XW * i + NP : XW * (i + 1)],
                    start=True, stop=True, skip_group_check=True,
                )
                nc.tensor.matmul(
                    p_rz[:, 0:NB], Wrzf[:], hf,
                    start=False, stop=True, skip_group_check=True,
                )
                nc.tensor.matmul(
                    p_rz[:, NB:NP], Wrzb[:], hb,
                    start=False, stop=True, skip_group_check=True,
                )
                nc.tensor.matmul(
                    p_n[:, 0:NP:2], Wnf[:], hnf,
                    start=True, stop=True, skip_group_check=True,
                )
                nc.tensor.matmul(
                    p_n[:, NP : 2 * NP : 2], Wnb[:], hnb,
                    start=True, stop=True, skip_group_check=True,
                )

                # ACT: r,z sigmoid then (after scan1) tanh
                nc.scalar.activation(
                    d0n[:, 1 : 2 * NP : 2], p_rz[:], AF.Sigmoid
                )
                # DVE, chain-first: scan1 right after sigmoid
                nc.vector.tensor_tensor_scan(
                    sc[:], d0n[0:H, :], p_n[:], 0.0, OP.mult, OP.add
                )
                nc.scalar.activation(
                    d1t[:, 0 : 2 * NP : 2], sc[:, 1 : 2 * NP : 2], AF.Tanh
                )
                # off-chain DVE while tanh runs
                nc.vector.tensor_copy(z0[:], d0n[H:128, 1 : 2 * NP : 2])
                nc.vector.tensor_scalar(
                    d0t[:, 1 : 2 * NP : 2], z0[:], 1.0, -1.0,
                    OP.subtract, OP.mult,
                )
                nc.vector.tensor_tensor(
                    d1t[:, 1 : 2 * NP : 2], z0[:],
                    Hist[0:H, 32 * i + 1 : 32 * i + 32 : 2], OP.mult,
                )
                # h' = (1-z)*n + z*h
                with nc.allow_low_precision(
                    reason="h state bf16; scan accumulates fp32 internally"
                ):
                    nc.vector.tensor_tensor_scan(
                        Hist[0:H, 32 * (i + 1) : 32 * (i + 2)],
                        d0t[:], d1t[:], 0.0, OP.mult, OP.add,
                    )
                # backward history in time order
                nc.vector.tensor_copy(
                    HistB[:, NB * (T - 1 - i) : NB * (T - i)],
                    Hist[0:H, 32 * (i + 1) + 17 : 32 * (i + 1) + 32 : 2],
                )

            with (
                tc.tile_pool(name="ps1", bufs=2, space="PSUM") as ps1,
                tc.tile_pool(name="ps2", bufs=2, space="PSUM") as ps2,
            ):
                stage1_chunk(ps1, 0)
                stage1_chunk(ps1, 1)
                for i in range(CS):
                    scan_step(ps2, i)
                stage1_chunk(ps1, 2)
                for i in range(CS, 2 * CS):
                    scan_step(ps2, i)
                stage1_chunk(ps1, 3)
                for i in range(2 * CS, T):
                    scan_step(ps2, i)

            # ---- stage 3: attention pooling + classifier
            Hist_v = Hist[0:H, :].rearrange("p (i c) -> p i c", c=32)
            s_sb = wp.tile([1, TB], F32)
            e_sb = wp.tile([1, TB], BF16)
            tmpf = wp.tile([H, TB], BF16)
            tmpb = wp.tile([H, TB], BF16)
            ctxf = wp.tile([H, NB], BF16)
            ctxb = wp.tile([H, NB], BF16)
            ctxf2 = wp.tile([H, NB * CH], BF16)
            ctxb2 = wp.tile([H, NB * CH], BF16)
            sum2 = wp.tile([1, NB * CH], F32)
            sums = wp.tile([1, NB], F32)
            inv = wp.tile([1, NB], F32)
            lraw = wp.tile([1, NB], F32)
            res = wp.tile([1, NB], F32)

            with tc.tile_pool(name="ps3", bufs=2, space="PSUM") as ps3:
                for ch in range(CH):
                    sp = ps3.tile([1, CW], F32)
                    rhs_f = Hist_v[:, 1 + ch * CS : 1 + (ch + 1) * CS, 1:16:2]
                    nc.tensor.matmul(
                        sp[:], attn_w[:, 0:1], rhs_f, start=True, stop=False,
                    )
                    nc.tensor.matmul(
                        sp[:], attn_w[:, 1:2],
                        HistB[:, ch * CW : (ch + 1) * CW],
                        start=False, stop=True,
                    )
                    nc.scalar.activation(
                        s_sb[:, ch * CW : (ch + 1) * CW], sp[:], AF.Tanh,
                        bias=ab_t[:],
                    )
                nc.scalar.activation(e_sb[:], s_sb[:], AF.Exp)

                e_v = e_sb[:].rearrange("p (c b t) -> p c b t", c=CH, b=NB)
                nc.vector.tensor_reduce(
                    sum2[:].rearrange("p (c b) -> p c b", c=CH),
                    e_v, mybir.AxisListType.X, OP.add,
                )
                nc.vector.tensor_reduce(
                    sums[:], sum2[:].rearrange("p (c b) -> p b c", c=CH),
                    mybir.AxisListType.X, OP.add,
                )
                nc.vector.reciprocal(inv[:], sums[:])

                with nc.allow_low_precision(
                    reason="bf16 ctx feeds a bf16 matmul; fp32 reduce internally"
                ):
                    for ch in range(CH):
                        erep = ps3.tile([H, CW], F32)
                        nc.tensor.matmul(
                            erep[:], ones1[:, 0:H],
                            e_sb[:, ch * CW : (ch + 1) * CW],
                            start=True, stop=True,
                        )
                        rhs_f = Hist_v[
                            :, 1 + ch * CS : 1 + (ch + 1) * CS, 1:16:2
                        ].rearrange("p t b -> p b t")
                        rhs_b = HistB[:, ch * CW : (ch + 1) * CW].rearrange(
                            "p (t b) -> p b t", b=NB
                        )
                        nc.vector.tensor_tensor(
                            tmpf[:, ch * CW : (ch + 1) * CW], rhs_f, erep[:],
                            OP.mult,
                        )
                        nc.vector.tensor_tensor(
                            tmpb[:, ch * CW : (ch + 1) * CW], rhs_b, erep[:],
                            OP.mult,
                        )
                        nc.vector.tensor_reduce(
                            ctxf2[:, ch * NB : (ch + 1) * NB],
                            tmpf[:, ch * CW : (ch + 1) * CW].rearrange(
                                "p (b t) -> p b t", b=NB
                            ),
                            mybir.AxisListType.X, OP.add,
                        )
                        nc.vector.tensor_reduce(
                            ctxb2[:, ch * NB : (ch + 1) * NB],
                            tmpb[:, ch * CW : (ch + 1) * CW].rearrange(
                                "p (b t) -> p b t", b=NB
                            ),
                            mybir.AxisListType.X, OP.add,
                        )
                    nc.vector.tensor_reduce(
                        ctxf[:], ctxf2[:].rearrange("p (c b) -> p b c", c=CH),
                        mybir.AxisListType.X, OP.add,
                    )
                    nc.vector.tensor_reduce(
                        ctxb[:], ctxb2[:].rearrange("p (c b) -> p b c", c=CH),
                        mybir.AxisListType.X, OP.add,
                    )

                pl = ps3.tile([1, NB], F32)
                nc.tensor.matmul(pl[:], clf_w[:, 0:1], ctxf[:], start=True, stop=False)
                nc.tensor.matmul(pl[:], clf_w[:, 1:2], ctxb[:], start=False, stop=True)
                nc.vector.tensor_tensor(lraw[:], pl[:], inv[:], OP.mult)
                # sigmoid via the already-loaded exp table set:
                # sigmoid(x + cb) = 1 / (1 + exp(-x - cb))
                e2 = wp.tile([1, NB], F32)
                nc.scalar.activation(e2[:], lraw[:], AF.Exp, bias=ncb_t[:], scale=-1.0)
                nc.vector.tensor_scalar(res[:], e2[:], 1.0, None, OP.add)
                nc.vector.reciprocal(res[:], res[:])
                nc.sync.dma_start(d_out[:], res[:])

    return nc


def _legalize_waits(nc, max_waits: int = 1):
    """This container's walrus build allows only one sync-wait slot per
    instruction. Hoist extra waits onto same-engine NoOps inserted right
    before the offending instruction (the sequencer honors them in order)."""
    from concourse import mybir

    ctr = 0
    for f in nc.m.functions:
        for blk in f.blocks:
            out = []
            changed = False
            for inst in blk.instructions:
                si = inst.sync_info
                waits = list(si.on_wait) if (si is not None and si.on_wait) else []
                if len(waits) > max_waits:
                    keep = waits[-max_waits:]
                    for w in waits[:-max_waits]:
                        ctr += 1
                        nop = mybir.InstNoOp(name=f"lwn-{ctr}", ins=[], outs=[])
                        nop.engine = inst.engine
                        nop.sync_info = mybir.SyncInfo(on_wait=[w], on_update=[])
                        out.append(nop)
                    inst.sync_info = mybir.SyncInfo(
                        on_wait=keep, on_update=list(si.on_update or [])
                    )
                    changed = True
                out.append(inst)
            if changed:
                blk.instructions = out
    return nc


def _get_nc(attn_b: float, clf_b: float):
    key = (attn_b, clf_b)
    if key not in _CACHE:
        _CACHE[key] = _legalize_waits(_build(attn_b, clf_b))
    return _CACHE[key]


def kernel(**inputs) -> np.ndarray:
    from concourse.bass_utils import run_bass_kernel_spmd

    in_maps, attn_b, clf_b = _fold(inputs)
    nc = _get_nc(attn_b, clf_b)
    res = run_bass_kernel_spmd(nc, in_maps, core_ids=list(range(NC)))
    out = np.empty((B, 1), np.float32)
    for c in range(NC):
        out[c * NB : (c + 1) * NB, 0] = res.results[c]["out"][0]
    return out


# revision 28
# speedup vs baseline: 1.0249x; 1.0249x over previous
"""Trainium2 Bass kernel for nn_DCGRU (EEG DCGRU: ChebConv+GCN -> biGRU ->
attention -> classifier).

Strategy:
  * Host-side algebraic fold: because F_IN=1, the entire front end
    (ChebConv + GCNConv + node-flatten + GRU input projection) collapses to
    one [192, 64] matrix per direction applied to x[b, :, t], plus a
    constant. This removes the 420MB gru_in intermediate exactly.
  * Data-parallel over batch: 8 cores x 8 batches.
  * Device: stage-1 matmuls (fp32) produce per-step gate preactivations,
    cast to bf16; a fused 200-step bidirectional GRU scan where all scan
    matmuls are bf16 single-pass (fp32 would run 2 LOW/HIGH passes on PE);
    one fused PE inject per step preloads the whole PSUM gate tile.
  * The scan's serial chain per step is mm -> sigmoid -> scan -> tanh ->
    scan; DVE instructions are emitted chain-first so in-order engines
    don't delay the critical path.
  * Stage-1 chunks are emitted interleaved with scan step groups so their
    PE work hides in the scan's idle PE slots.
"""

import numpy as np
from ml_dtypes import bfloat16

N = 64
T = 200
B = 64
H = 64
NC = 8
NB = B // NC          # batches per core
NP = 2 * NB           # scan pair-columns per step (fwd 0:8, bwd 8:16)
TB = T * NB           # stage-1 columns (t, b)
XW = 2 * NP           # 32: per-step Xall columns (16 rz + 16 xn)

_CACHE = {}


def _layout():
    """fp32 blob column layout."""
    off = 0
    bo = {}
    for name, w in (("ab", 1), ("ncb", 1)):
        bo[name] = off
        off += w
    return bo, off


def _layoutx():
    """bf16 x blob column layout."""
    off = 0
    bo = {}
    for name, w in (("xf", TB), ("xb", TB)):
        bo[name] = off
        off += w
    return bo, off


def _layout16():
    """bf16 blob column layout."""
    off = 0
    bo = {}
    for name, w in (
        ("ident", 128), ("Wrzf", 2 * H), ("Wrzb", 2 * H),
        ("Wnf", H), ("Wnb", H), ("attn", 2), ("clf", 2), ("ones", 128),
        ("MfT", 3 * H), ("MbT", 3 * H),
    ):
        bo[name] = off
        off += w
    return bo, off


# --------------------------------------------------------------------------
# host-side fold
# --------------------------------------------------------------------------

def _fold_direction(Wih, bih, Whh, bhh, L, Sg, Wcheb, bcheb, Wgcn, bgcn):
    f64 = np.float64
    Wr = Wih.astype(f64).reshape(3 * H, N, 2 * H)
    Wc = Wr[:, :, 0:H]          # cheb half
    Wg_ = Wr[:, :, H : 2 * H]   # gcn half
    A0 = np.einsum("gnc,c->gn", Wc, Wcheb[0, 0].astype(f64))
    A1 = np.einsum("gnc,c->gn", Wc, Wcheb[1, 0].astype(f64))
    A2 = np.einsum("gnc,c->gn", Wc, Wcheb[2, 0].astype(f64))
    Ag = np.einsum("gnc,c->gn", Wg_, Wgcn[:, :].astype(f64)[0])
    M = A0 + A1 @ L + A2 @ (2.0 * (L @ L) - np.eye(N)) + Ag @ Sg
    cst = (
        np.einsum("gnc,c->g", Wc, bcheb.astype(f64))
        + np.einsum("gnc,c->g", Wg_, bgcn.astype(f64))
        + bih.astype(f64)
    )
    cfull = cst.copy()
    cfull[0 : 2 * H] += bhh.astype(f64)[0 : 2 * H]   # r,z recurrent biases
    MT_aug = np.vstack([M.T, cfull[None, :]]).astype(np.float32)       # [65,192]
    WhT_rz = np.ascontiguousarray(Whh[0 : 2 * H, :].T).astype(np.float32)  # [64,128]
    WhT_n = np.vstack(
        [Whh[2 * H : 3 * H, :].T, bhh[2 * H : 3 * H][None, :]]
    ).astype(np.float32)                                                # [65,64]
    return MT_aug, WhT_rz, WhT_n


def _fold(inputs):
    f64 = np.float64
    # ChebConv normalized operator (PyG sym norm, lambda_max=2)
    row, col = np.asarray(inputs["spatial_ei"][0]), np.asarray(inputs["spatial_ei"][1])
    ew = np.asarray(inputs["spatial_ew"]).astype(f64)
    deg = np.zeros(N, f64)
    np.add.at(deg, row, ew)
    dinv = np.where(deg > 0, 1.0 / np.sqrt(np.where(deg > 0, deg, 1.0)), 0.0)
    wn = dinv[row] * ew * dinv[col]
    S = np.zeros((N, N), f64)
    np.add.at(S, (col, row), wn)
    L = -S

    # GCNConv operator (gcn_norm with self loops, weight 1)
    row, col = (
        np.asarray(inputs["functional_ei"][0]),
        np.asarray(inputs["functional_ei"][1]),
    )
    ew = np.asarray(inputs["functional_ew"]).astype(f64)
    deg = np.zeros(N, f64)
    np.add.at(deg, col, ew)
    deg += 1.0
    dinv = 1.0 / np.sqrt(deg)
    wn = dinv[row] * ew * dinv[col]
    Sg = np.zeros((N, N), f64)
    np.add.at(Sg, (col, row), wn)
    Sg[np.arange(N), np.arange(N)] += dinv * dinv

    Wcheb = np.asarray(inputs["Wcheb"])
    bcheb = np.asarray(inputs["bcheb"])
    Wgcn = np.asarray(inputs["Wgcn"])
    bgcn = np.asarray(inputs["bgcn"])

    MfT, WhT_rz_f, WhT_n_f = _fold_direction(
        np.asarray(inputs["Wih_f"]), np.asarray(inputs["bih_f"]),
        np.asarray(inputs["Whh_f"]), np.asarray(inputs["bhh_f"]),
        L, Sg, Wcheb, bcheb, Wgcn, bgcn,
    )
    MbT, WhT_rz_b, WhT_n_b = _fold_direction(
        np.asarray(inputs["Wih_b"]), np.asarray(inputs["bih_b"]),
        np.asarray(inputs["Whh_b"]), np.asarray(inputs["bhh_b"]),
        L, Sg, Wcheb, bcheb, Wgcn, bgcn,
    )

    attn_W = np.asarray(inputs["attn_W"]).astype(np.float32)
    clf_W = np.asarray(inputs["clf_W"]).astype(np.float32)
    attn_w2 = np.ascontiguousarray(np.stack([attn_W[0:H, 0], attn_W[H : 2 * H, 0]], 1))
    clf_w2 = np.ascontiguousarray(np.stack([clf_W[0:H, 0], clf_W[H : 2 * H, 0]], 1))
    attn_b = float(np.asarray(inputs["attn_b"]).reshape(-1)[0])
    clf_b = float(np.asarray(inputs["clf_b"]).reshape(-1)[0])

    BO, CB = _layout()
    base = np.zeros((128, CB), np.float32)
    base[0, BO["ab"]] = attn_b
    base[0, BO["ncb"]] = -clf_b

    B16, CB16 = _layout16()
    b16 = np.zeros((128, CB16), np.float32)
    b16[0:128, B16["ident"] : B16["ident"] + 128] = np.eye(128)
    b16[0:H, B16["Wrzf"] : B16["Wrzf"] + 2 * H] = WhT_rz_f
    b16[0:H, B16["Wrzb"] : B16["Wrzb"] + 2 * H] = WhT_rz_b
    b16[0 : H + 1, B16["Wnf"] : B16["Wnf"] + H] = WhT_n_f
    b16[0 : H + 1, B16["Wnb"] : B16["Wnb"] + H] = WhT_n_b
    b16[0:H, B16["attn"] : B16["attn"] + 2] = attn_w2
    b16[0:H, B16["clf"] : B16["clf"] + 2] = clf_w2
    b16[0:1, B16["ones"] : B16["ones"] + 128] = 1.0
    b16[0 : N + 1, B16["MfT"] : B16["MfT"] + 3 * H] = MfT
    b16[0 : N + 1, B16["MbT"] : B16["MbT"] + 3 * H] = MbT
    b16 = b16.astype(bfloat16)
    ones_row = np.ones((1, 32 * (T + 1)), dtype=bfloat16)

    x = np.asarray(inputs["x"]).astype(np.float32)
    XO, CBX = _layoutx()
    in_maps = []
    for c in range(NC):
        xc = x[c * NB : (c + 1) * NB]                       # [NB, N, T]
        xblob = np.zeros((128, CBX), np.float32)
        xblob[0:N, XO["xf"] : XO["xf"] + TB] = xc.transpose(1, 2, 0).reshape(N, TB)
        xblob[N, XO["xf"] : XO["xf"] + TB] = 1.0
        xblob[0:N, XO["xb"] : XO["xb"] + TB] = (
            xc[:, :, ::-1].transpose(1, 2, 0).reshape(N, TB)
        )
        xblob[N, XO["xb"] : XO["xb"] + TB] = 1.0
        xb16 = xblob.astype(bfloat16)
        in_maps.append({
            "blob": base, "blob16": b16,
            "xblobf": np.ascontiguousarray(xb16[:, XO["xf"] : XO["xf"] + TB]),
            "xblobb": np.ascontiguousarray(xb16[:, XO["xb"] : XO["xb"] + TB]),
            "ones": ones_row,
        })
    return in_maps, attn_b, clf_b


# --------------------------------------------------------------------------
# device program
# --------------------------------------------------------------------------

def _build(attn_b: float, clf_b: float):
    import concourse.bass as bass
    import concourse.tile as tile
    from concourse import mybir

    F32 = mybir.dt.float32
    BF16 = mybir.dt.bfloat16
    AF = mybir.ActivationFunctionType
    OP = mybir.AluOpType

    nc = bass.Bass()

    BO, CB = _layout()
    B16, CB16 = _layout16()
    XO, CBX = _layoutx()
    d_blob = nc.declare_dram_parameter("blob", [128, CB], F32, isOutput=False)
    d_b16 = nc.declare_dram_parameter("blob16", [128, CB16], BF16, isOutput=False)
    d_xf = nc.declare_dram_parameter("xblobf", [128, TB], BF16, isOutput=False)
    d_xb = nc.declare_dram_parameter("xblobb", [128, TB], BF16, isOutput=False)
    d_ones = nc.declare_dram_parameter("ones", [1, 32 * (T + 1)], BF16,
                                       isOutput=False)
    d_out = nc.declare_dram_parameter("out", [1, NB], F32, isOutput=True)

    CH = 4                 # stage-1 / attention chunks
    CW = TB // CH          # 400 columns per chunk
    CS = T // CH           # 50 steps per chunk

    with tile.TileContext(nc) as tc:
        with (
            tc.tile_pool(name="const", bufs=1) as cp,
            tc.tile_pool(name="work", bufs=1) as wp,
        ):
            # ---- persistent SBUF tiles
            blob = cp.tile([128, CB], F32)
            b16 = cp.tile([128, CB16], BF16)
            xtf = cp.tile([128, TB], BF16)
            xtb = cp.tile([128, TB], BF16)
            xf = xtf[0 : N + 1, 0:TB]
            xb = xtb[0 : N + 1, 0:TB]
            MfT = b16[0 : N + 1, B16["MfT"] : B16["MfT"] + 3 * H]
            MbT = b16[0 : N + 1, B16["MbT"] : B16["MbT"] + 3 * H]
            ab_t = blob[0:1, BO["ab"] : BO["ab"] + 1]
            ncb_t = blob[0:1, BO["ncb"] : BO["ncb"] + 1]
            Xall = cp.tile([128, XW * T], BF16)
            Hist = cp.tile([H + 1, 32 * (T + 1)], BF16)
            HistB = cp.tile([H, NB * T], BF16)
            ident = b16[0:128, B16["ident"] : B16["ident"] + 128]
            Wrzf = b16[0:H, B16["Wrzf"] : B16["Wrzf"] + 2 * H]
            Wrzb = b16[0:H, B16["Wrzb"] : B16["Wrzb"] + 2 * H]
            Wnf = b16[0 : H + 1, B16["Wnf"] : B16["Wnf"] + H]
            Wnb = b16[0 : H + 1, B16["Wnb"] : B16["Wnb"] + H]
            attn_w = b16[0:H, B16["attn"] : B16["attn"] + 2]
            clf_w = b16[0:H, B16["clf"] : B16["clf"] + 2]
            ones1 = b16[0:1, B16["ones"] : B16["ones"] + 128]

            d0n = cp.tile([128, 2 * NP], F32)      # (0 | r) rows 0:64; (. | z) 64:128
            d0t = cp.tile([H, 2 * NP], F32)        # (0 | 1-z)
            d1t = cp.tile([H, 2 * NP], F32)        # (n | z*h)

            ones_n = wp.tile([1, 128], F32)
            warm16 = wp.tile([H, 128], BF16)

            nc.sync.dma_start(b16[:], d_b16[:])
            nc.sync.dma_start(blob[:], d_blob[:])
            nc.sync.dma_start(xtf[:], d_xf[:])
            nc.gpsimd.dma_start(xtb[:], d_xb[:])
            nc.scalar.dma_start(Hist[H : H + 1, :], d_ones[:])

            # warm the PE HAM clock gate during the DMA: ~5us of matmuls
            # on a scratch psum (never read)
            nc.vector.memset(warm16[:], 1.0)
            tblw = wp.tile([1, 16], F32)
            nc.scalar.activation(tblw[:], warm16[0:1, 0:16], AF.Sigmoid)
            with tc.tile_pool(name="warm", bufs=1, space="PSUM") as pw:
                wps = pw.tile([128, 128], F32)
                for _ in range(14):
                    nc.tensor.matmul(
                        wps[:], warm16[:], warm16[:],
                        start=True, stop=True, skip_group_check=True,
                    )

            nc.vector.memset(Hist[0:H, 0:32], 0.0)
            nc.vector.memset(d0n[:], 0.0)
            nc.vector.memset(d0t[:], 0.0)
            nc.vector.memset(d1t[:], 0.0)
            nc.vector.memset(ones_n[:], 1.0)

            X3 = Xall[:].rearrange("p (i c) -> p i c", c=XW)

            # ---- stage 1 (fp32 math, bf16 out), emitted per-chunk so it can
            # interleave with the scan
            def stage1_chunk(ps1, ch):
                for xa, MT, off in ((xf, MfT, 0), (xb, MbT, NB)):
                    prz = ps1.tile([128, CW], F32)
                    pn = ps1.tile([H, CW], F32)
                    nc.tensor.matmul(
                        prz[0:H, :], MT[:, 0:H],
                        xa[:, ch * CW : (ch + 1) * CW],
                        start=True, stop=True,
                    )
                    nc.tensor.matmul(
                        prz[H:128, :], MT[:, H : 2 * H],
                        xa[:, ch * CW : (ch + 1) * CW],
                        start=True, stop=True, tile_position=(0, 64),
                    )
                    nc.tensor.matmul(
                        pn[:], MT[:, 2 * H : 3 * H],
                        xa[:, ch * CW : (ch + 1) * CW],
                        start=True, stop=True,
                    )
                    src_rz = prz[:].rearrange("p (i c) -> p i c", c=NB)
                    dst_rz = X3[:, ch * CS : (ch + 1) * CS, off : off + NB]
                    nc.vector.tensor_copy(dst_rz, src_rz)
                    src_n = pn[:].rearrange("p (i c) -> p i c", c=NB)
                    dst_n = X3[
                        0:H, ch * CS : (ch + 1) * CS,
                        NP + 2 * off + 1 : NP + 2 * off + 2 * NB : 2,
                    ]
                    nc.vector.tensor_copy(dst_n, src_n)

            # ---- one GRU scan step
            def scan_step(ps2, i):
                hf = Hist[0:H, 32 * i + 1 : 32 * i + 16 : 2]
                hb = Hist[0:H, 32 * i + 17 : 32 * i + 32 : 2]
                hnf = Hist[0 : H + 1, 32 * i + 1 : 32 * i + 16 : 2]
                hnb = Hist[0 : H + 1, 32 * i + 17 : 32 * i + 32 : 2]

                p_rz = ps2.tile([128, NP], F32)
                pnsc = ps2.tile([H, 4 * NP], F32)
                p_n = pnsc[:, 0 : 2 * NP]
                sc = pnsc[:, 2 * NP : 4 * NP]

                nc.tensor.matmul(
                    p_rz[:], ident[:], Xall[:, XW * i : XW * i + NP],
                    start=True, stop=False, skip_group_check=True,
                )
                nc.tensor.matmul(
                    p_n[:, 1 : 2 * NP : 2], ident[0:H, 0:H],
                    Xall[0:H, XW * i + NP : XW * (i + 1)],
                    start=True, stop=True, skip_group_check=True,
                )
                nc.tensor.matmul(
                    p_rz[:, 0:NB], Wrzf[:], hf,
                    start=False, stop=True, skip_group_check=True,
                )
                nc.tensor.matmul(
                    p_rz[:, NB:NP], Wrzb[:], hb,
                    start=False, stop=True, skip_group_check=True,
                )
                nc.tensor.matmul(
                    p_n[:, 0:NP:2], Wnf[:], hnf,
                    start=True, stop=True, skip_group_check=True,
                )
                nc.tensor.matmul(
                    p_n[:, NP : 2 * NP : 2], Wnb[:], hnb,
                    start=True, stop=True, skip_group_check=True,
                )

                # ACT: r,z sigmoid then (after scan1) tanh
                nc.scalar.activation(
                    d0n[:, 1 : 2 * NP : 2], p_rz[:], AF.Sigmoid
                )
                # DVE, chain-first: scan1 right after sigmoid
                nc.vector.tensor_tensor_scan(
                    sc[:], d0n[0:H, :], p_n[:], 0.0, OP.mult, OP.add
                )
                nc.scalar.activation(
                    d1t[:, 0 : 2 * NP : 2], sc[:, 1 : 2 * NP : 2], AF.Tanh
                )
                # off-chain DVE while tanh runs
                nc.vector.tensor_copy(z0[:], d0n[H:128, 1 : 2 * NP : 2])
                nc.vector.tensor_scalar(
                    d0t[:, 1 : 2 * NP : 2], z0[:], 1.0, -1.0,
                    OP.subtract, OP.mult,
                )
                nc.vector.tensor_tensor(
                    d1t[:, 1 : 2 * NP : 2], z0[:],
                    Hist[0:H, 32 * i + 1 : 32 * i + 32 : 2], OP.mult,
                )
                # h' = (1-z)*n + z*h
                with nc.allow_low_precision(
                    reason="h state bf16; scan accumulates fp32 internally"
                ):
                    nc.vector.tensor_tensor_scan(
                        Hist[0:H, 32 * (i + 1) : 32 * (i + 2)],
                        d0t[:], d1t[:], 0.0, OP.mult, OP.add,
                    )
                # backward history in time order
                nc.vector.tensor_copy(
                    HistB[:, NB * (T - 1 - i) : NB * (T - i)],
                    Hist[0:H, 32 * (i + 1) + 17 : 32 * (i + 1) + 32 : 2],
                )

            with (
                tc.tile_pool(name="ps1", bufs=2, space="PSUM") as ps1,
                tc.tile_pool(name="ps2", bufs=2, space="PSUM") as ps2,
            ):
                stage1_chunk(ps1, 0)
                stage1_chunk(ps1, 1)
                for i in range(CS):
                    scan_step(ps2, i)
                stage1_chunk(ps1, 2)
                for i in range(CS, 2 * CS):
                    scan_step(ps2, i)
                stage1_chunk(ps1, 3)
                for i in range(2 * CS, T):
                    scan_step(ps2, i)

            # ---- stage 3: attention pooling + classifier
            Hist_v = Hist[0:H, :].rearrange("p (i c) -> p i c", c=32)
            s_sb = wp.tile([1, TB], F32)
            e_sb = wp.tile([1, TB], BF16)
            tmpf = wp.tile([H, TB], BF16)
            tmpb = wp.tile([H, TB], BF16)
            ctxf = wp.tile([H, NB], BF16)
            ctxb = wp.tile([H, NB], BF16)
            ctxf2 = wp.tile([H, NB * CH], BF16)
            ctxb2 = wp.tile([H, NB * CH], BF16)
            sum2 = wp.tile([1, NB * CH], F32)
            sums = wp.tile([1, NB], F32)
            inv = wp.tile([1, NB], F32)
            lraw = wp.tile([1, NB], F32)
            res = wp.tile([1, NB], F32)

            with tc.tile_pool(name="ps3", bufs=2, space="PSUM") as ps3:
                for ch in range(CH):
                    sp = ps3.tile([1, CW], F32)
                    rhs_f = Hist_v[:, 1 + ch * CS : 1 + (ch + 1) * CS, 1:16:2]
                    nc.tensor.matmul(
                        sp[:], attn_w[:, 0:1], rhs_f, start=True, stop=False,
                    )
                    nc.tensor.matmul(
                        sp[:], attn_w[:, 1:2],
                        HistB[:, ch * CW : (ch + 1) * CW],
                        start=False, stop=True,
                    )
                    nc.scalar.activation(
                        s_sb[:, ch * CW : (ch + 1) * CW], sp[:], AF.Tanh,
                        bias=ab_t[:],
                    )
                nc.scalar.activation(e_sb[:], s_sb[:], AF.Exp)

                e_v = e_sb[:].rearrange("p (c b t) -> p c b t", c=CH, b=NB)
                nc.vector.tensor_reduce(
                    sum2[:].rearrange("p (c b) -> p c b", c=CH),
                    e_v, mybir.AxisListType.X, OP.add,
                )
                nc.vector.tensor_reduce(
                    sums[:], sum2[:].rearrange("p (c b) -> p b c", c=CH),
                    mybir.AxisListType.X, OP.add,
                )
                nc.vector.reciprocal(inv[:], sums[:])

                with nc.allow_low_precision(
                    reason="bf16 ctx feeds a bf16 matmul; fp32 reduce internally"
                ):
                    for ch in range(CH):
                        erep = ps3.tile([H, CW], F32)
                        nc.tensor.matmul(
                            erep[:], ones1[:, 0:H],
                            e_sb[:, ch * CW : (ch + 1) * CW],
                            start=True, stop=True,
                        )
                        rhs_f = Hist_v[
                            :, 1 + ch * CS : 1 + (ch + 1) * CS, 1:16:2
                        ].rearrange("p t b -> p b t")
                        rhs_b = HistB[:, ch * CW : (ch + 1) * CW].rearrange(
                            "p (t b) -> p b t", b=NB
                        )
                        nc.vector.tensor_tensor(
                            tmpf[:, ch * CW : (ch + 1) * CW], rhs_f, erep[:],
                            OP.mult,
                        )
                        nc.vector.tensor_tensor(
                            tmpb[:, ch * CW : (ch + 1) * CW], rhs_b, erep[:],
                            OP.mult,
                        )
                        nc.vector.tensor_reduce(
                            ctxf2[:, ch * NB : (ch + 1) * NB],
                            tmpf[:, ch * CW : (ch + 1) * CW].rearrange(
                                "p (b t) -> p b t", b=NB
                            ),
                            mybir.AxisListType.X, OP.add,
                        )
                        nc.vector.tensor_reduce(
                            ctxb2[:, ch * NB : (ch + 1) * NB],
                            tmpb[:, ch * CW : (ch + 1) * CW].rearrange(
                                "p (b t) -> p b t", b=NB
                            ),
                            mybir.AxisListType.X, OP.add,
                        )
                    nc.vector.tensor_reduce(
                        ctxf[:], ctxf2[:].rearrange("p (c b) -> p b c", c=CH),
                        mybir.AxisListType.X, OP.add,
                    )
                    nc.vector.tensor_reduce(
                        ctxb[:], ctxb2[:].rearrange("p (c b) -> p b c", c=CH),
                        mybir.AxisListType.X, OP.add,
                    )

                pl = ps3.tile([1, NB], F32)
                nc.tensor.matmul(pl[:], clf_w[:, 0:1], ctxf[:], start=True, stop=False)
                nc.tensor.matmul(pl[:], clf_w[:, 1:2], ctxb[:], start=False, stop=True)
                nc.vector.tensor_tensor(lraw[:], pl[:], inv[:], OP.mult)
                # sigmoid via the already-loaded exp table set:
                # sigmoid(x + cb) = 1 / (1 + exp(-x - cb))
                e2 = wp.tile([1, NB], F32)
                nc.scalar.activation(e2[:], lraw[:], AF.Exp, bias=ncb_t[:], scale=-1.0)
                nc.vector.tensor_scalar(res[:], e2[:], 1.0, None, OP.add)
                nc.vector.reciprocal(res[:], res[:])
                nc.sync.dma_start(d_out[:], res[:])

    return nc


def _legalize_waits(nc, max_waits: int = 1):
    """This container's walrus build allows only one sync-wait slot per
    instruction. Hoist extra waits onto same-engine NoOps inserted right
    before the offending instruction (the sequencer honors them in order)."""
    from concourse import mybir

    ctr = 0
    for f in nc.m.functions:
        for blk in f.blocks:
            out = []
            changed = False
            for inst in blk.instructions:
                si = inst.sync_info
                waits = list(si.on_wait) if (si is not None and si.on_wait) else []
                if len(waits) > max_waits:
                    keep = waits[-max_waits:]
                    for w in waits[:-max_waits]:
                        ctr += 1
                        nop = mybir.InstNoOp(name=f"lwn-{ctr}", ins=[], outs=[])
                        nop.engine = inst.engine
                        nop.sync_info = mybir.SyncInfo(on_wait=[w], on_update=[])
                        out.append(nop)
                    inst.sync_info = mybir.SyncInfo(
                        on_wait=keep, on_update=list(si.on_update or [])
                    )
                    changed = True
                out.append(inst)
            if changed:
                blk.instructions = out
    return nc


def _get_nc(attn_b: float, clf_b: float):
    key = (attn_b, clf_b)
    if key not in _CACHE:
        _CACHE[key] = _legalize_waits(_build(attn_b, clf_b))
    return _CACHE[key]


def kernel(**inputs) -> np.ndarray:
    from concourse.bass_utils import run_bass_kernel_spmd

    in_maps, attn_b, clf_b = _fold(inputs)
    nc = _get_nc(attn_b, clf_b)
    res = run_bass_kernel_spmd(nc, in_maps, core_ids=list(range(NC)))
    out = np.empty((B, 1), np.float32)
    for c in range(NC):
        out[c * NB : (c + 1) * NB, 0] = res.results[c]["out"][0]
    return out
